# revision 6
# baseline (speedup 1.0000x reference)
"""Trainium2 Bass kernel for nn_AttentionBlock (GroupNorm + 8-head self-attention
+ projection + residual) on input x:(8,512,32,32) f32.

Strategy: pure data-parallel over batch — each of the 8 NeuronCores processes
one batch element end-to-end (no collectives). Per core:

  x (512,1024) --GroupNorm--> xn --qkv matmul--> Q,K (o-part,t-free), V^T (s-part,c-free)
  per head h: S^T = K_h^T Q_h (s-part, t-free); expS = exp(S^T) [no max-sub; |S|<7]
              H_ext = [V_h^T | 1]^T expS  (rows 0..63 = unnormalized AV, row 64 = softmax denom)
              H = H_ext[0:64] * recip(H_ext[64]) broadcast
  out = proj @ H + bproj + x

Matmuls run as float32r (full-rate fp32 on the PE array); attention weights
(expS) and V^T are bf16. Softmax scale and the q/k head split/scale are folded
into the qkv weights on the host, which also pre-transposes all weights.
"""

import math

import numpy as np

import concourse.bacc as bacc
import concourse.bass as bass
import concourse.bass2jax as bass2jax
import concourse.mybir as mybir
import concourse.tile as tile
from concourse.bass_utils import run_bass_kernel_spmd


def _install_neff_disk_cache():
    """Wrap bass2jax.neuronx_cc_hook with a content-addressed on-disk cache so
    repeated processes skip the multi-minute walrus compile of an identical
    kernel. Keyed on the HLO bytes (which embed the serialized BIR)."""
    if getattr(bass2jax, "_ant_neff_disk_cache", False):
        return
    import hashlib
    import os

    cache_dir = os.environ.get("BASS_NEFF_CACHE", "/tmp/bass_neff_cache")
    try:
        os.makedirs(cache_dir, exist_ok=True)
    except OSError:
        return
    orig = bass2jax.neuronx_cc_hook

    def cached_hook(code, code_format, platform_version, file_prefix):
        key = None
        if isinstance(code, (bytes, bytearray)) and b"bass_exec" in code:
            key = hashlib.sha256(bytes(code)).hexdigest()
            path = os.path.join(cache_dir, key + ".neffcc")
            if os.path.exists(path):
                with open(path, "rb") as f:
                    return 0, f.read()
        r = orig(code, code_format, platform_version, file_prefix)
        if key is not None:
            try:
                status, data = r
                if status == 0 and isinstance(data, (bytes, bytearray)):
                    tmp = path + f".tmp{os.getpid()}"
                    with open(tmp, "wb") as f:
                        f.write(bytes(data))
                    os.replace(tmp, path)
            except Exception:
                pass
        return r

    bass2jax.neuronx_cc_hook = cached_hook
    bass2jax._ant_neff_disk_cache = True


_install_neff_disk_cache()

B = 8
C = 512
T = 1024
HEADS = 8
HD = 64  # head dim
G = 32  # groupnorm groups
GSIZE = C // G  # 16 channels per group
EPS = 1e-5

F32 = mybir.dt.float32
F32R = mybir.dt.float32r
BF16 = mybir.dt.bfloat16
AX = mybir.AxisListType
ALU = mybir.AluOpType
ACTF = mybir.ActivationFunctionType


def _emit(nc: bacc.Bacc) -> None:
    x_d = nc.dram_tensor("x", [C, T], F32, kind="ExternalInput").ap()
    wqkvT_d = nc.dram_tensor("wqkvT", [C, 3 * C], BF16, kind="ExternalInput").ap()
    bqk_d = nc.dram_tensor("bqk", [2 * C, 1], F32, kind="ExternalInput").ap()
    bvrow_d = nc.dram_tensor("bv_row", [1, C], F32, kind="ExternalInput").ap()
    gnw_d = nc.dram_tensor("gnw", [C, 1], F32, kind="ExternalInput").ap()
    gnb_d = nc.dram_tensor("gnb", [C, 1], F32, kind="ExternalInput").ap()
    wprojT_d = nc.dram_tensor("wprojT", [C, C], BF16, kind="ExternalInput").ap()
    bproj_d = nc.dram_tensor("bproj", [C, 1], F32, kind="ExternalInput").ap()
    gmat_d = nc.dram_tensor("gmat", [4, 128, G], F32, kind="ExternalInput").ap()
    gmatT_d = nc.dram_tensor("gmatT", [4, G, 128], F32, kind="ExternalInput").ap()
    out_d = nc.dram_tensor("out", [C, T], F32, kind="ExternalOutput").ap()

    x_r = x_d.rearrange("(j p) t -> j p t", p=128)
    wqkvT_r = wqkvT_d.rearrange("(j p) o -> j p o", p=128)
    bqk_r = bqk_d.rearrange("(j p) o -> j p o", p=128)
    gnw_r = gnw_d.rearrange("(j p) o -> j p o", p=128)
    gnb_r = gnb_d.rearrange("(j p) o -> j p o", p=128)
    wprojT_r = wprojT_d.rearrange("(j p) o -> j p o", p=128)
    bproj_r = bproj_d.rearrange("(j p) o -> j p o", p=128)
    out_r = out_d.rearrange("(j p) t -> j p t", p=128)

    with tile.TileContext(nc) as tc:
        with (
            tc.tile_pool(name="persist", bufs=1) as pp,
            tc.tile_pool(name="work", bufs=2) as wp,
            tc.tile_pool(name="psum", bufs=1, space="PSUM") as pool_ps,
        ):
            # ---- constant / weight loads ----
            xt = []
            for j in range(4):
                x_sb = pp.tile([128, T], F32, name=f"x{j}", tag=f"x{j}")
                nc.sync.dma_start(out=x_sb, in_=x_r[j])
                xt.append(x_sb)
            wqkvT = []
            for j in range(4):
                wq_sb = pp.tile([128, 3 * C], BF16, name=f"wqkvT{j}", tag=f"wqkvT{j}")
                nc.sync.dma_start(out=wq_sb, in_=wqkvT_r[j])
                wqkvT.append(wq_sb)
            wprojT = []
            for j in range(4):
                wp_sb = pp.tile([128, C], BF16, name=f"wprojT{j}", tag=f"wprojT{j}")
                nc.sync.dma_start(out=wp_sb, in_=wprojT_r[j])
                wprojT.append(wp_sb)
            bqk = []
            for j in range(8):
                b_sb = pp.tile([128, 1], F32, name=f"bqk{j}", tag=f"bqk{j}")
                nc.sync.dma_start(out=b_sb, in_=bqk_r[j])
                bqk.append(b_sb)
            bproj = []
            for j in range(4):
                bp_sb = pp.tile([128, 1], F32, name=f"bproj{j}", tag=f"bproj{j}")
                nc.sync.dma_start(out=bp_sb, in_=bproj_r[j])
                bproj.append(bp_sb)
            gnw, gnb = [], []
            for j in range(4):
                gw_sb = pp.tile([128, 1], F32, name=f"gnw{j}", tag=f"gnw{j}")
                nc.sync.dma_start(out=gw_sb, in_=gnw_r[j])
                gnw.append(gw_sb)
                gb_sb = pp.tile([128, 1], F32, name=f"gnb{j}", tag=f"gnb{j}")
                nc.sync.dma_start(out=gb_sb, in_=gnb_r[j])
                gnb.append(gb_sb)
            gmat, gmatT = [], []
            for j in range(4):
                gm_sb = pp.tile([128, G], F32, name=f"gmat{j}", tag=f"gmat{j}")
                nc.sync.dma_start(out=gm_sb, in_=gmat_d[j])
                gmat.append(gm_sb)
                gmt_sb = pp.tile([G, 128], F32, name=f"gmatT{j}", tag=f"gmatT{j}")
                nc.sync.dma_start(out=gmt_sb, in_=gmatT_d[j])
                gmatT.append(gmt_sb)
            bvrow = pp.tile([1, C], F32, name="bvrow", tag="bvrow")
            nc.sync.dma_start(out=bvrow, in_=bvrow_d)
            bvb = pp.tile([128, C], F32, name="bvb", tag="bvb")
            nc.gpsimd.partition_broadcast(out_ap=bvb, in_ap=bvrow, channels=128)

            # ---- GroupNorm statistics ----
            stats = []
            for j in range(4):
                stat = pp.tile([128, 2], F32, name=f"stat{j}", tag=f"stat{j}")
                nc.vector.reduce_sum(stat[:, 0:1], xt[j], axis=AX.X)
                scr = wp.tile([128, T], F32, name="sqscr", tag="sqscr", bufs=1)
                nc.scalar.activation(
                    out=scr, in_=xt[j], func=ACTF.Square, accum_out=stat[:, 1:2]
                )
                stats.append(stat)
            gsum = pool_ps.tile([G, 2], F32, name="gsum", tag="sm", bufs=4)
            for j in range(4):
                nc.tensor.matmul(
                    out=gsum, lhsT=gmat[j], rhs=stats[j], start=(j == 0), stop=(j == 3)
                )
            gstat = pp.tile([G, 2], F32, name="gstat", tag="gstat")
            nc.vector.tensor_scalar_mul(gstat, gsum, 1.0 / float(GSIZE * T))
            m2 = pp.tile([G, 1], F32, name="m2", tag="m2")
            nc.vector.tensor_tensor(out=m2, in0=gstat[:, 0:1], in1=gstat[:, 0:1], op=ALU.mult)
            var = pp.tile([G, 1], F32, name="var", tag="var")
            nc.vector.tensor_tensor(out=var, in0=gstat[:, 1:2], in1=m2, op=ALU.subtract)
            nc.vector.tensor_scalar_add(var, var, EPS)
            std = pp.tile([G, 1], F32, name="std", tag="std")
            nc.scalar.activation(out=std, in_=var, func=ACTF.Sqrt, bias=0.0)
            grs = pp.tile([G, 2], F32, name="grs", tag="grs")
            nc.vector.tensor_copy(out=grs[:, 0:1], in_=gstat[:, 0:1])
            nc.vector.reciprocal(out=grs[:, 1:2], in_=std)

            xn = []
            for j in range(4):
                chs = pool_ps.tile([128, 2], F32, name=f"chs{j}", tag="sm", bufs=4)
                nc.tensor.matmul(out=chs, lhsT=gmatT[j], rhs=grs, start=True, stop=True)
                a_j = pp.tile([128, 1], F32, name=f"a{j}", tag=f"a{j}")
                nc.vector.tensor_tensor(out=a_j, in0=gnw[j], in1=chs[:, 1:2], op=ALU.mult)
                nb = wp.tile([128, 1], F32, name="nb", tag="nb")
                nc.vector.tensor_tensor(out=nb, in0=chs[:, 0:1], in1=a_j, op=ALU.mult)
                b_j = pp.tile([128, 1], F32, name=f"b{j}", tag=f"b{j}")
                nc.vector.tensor_tensor(out=b_j, in0=gnb[j], in1=nb, op=ALU.subtract)
                xn_j = pp.tile([128, T], BF16, name=f"xn{j}", tag=f"xn{j}")
                nc.vector.tensor_scalar(
                    out=xn_j, in0=xt[j], scalar1=a_j, scalar2=b_j, op0=ALU.mult, op1=ALU.add
                )
                xn.append(xn_j)

            # ---- V^T (+ ones column), all heads: vT[s] = (128 s, 8 heads, 65) bf16 ----
            vT = []
            for s in range(8):
                vt_s = pp.tile([128, HEADS, HD + 1], BF16, name=f"vT{s}", tag=f"vT{s}")
                nc.vector.memset(vt_s[:, :, HD : HD + 1], 1.0)
                vps = pool_ps.tile([128, C], F32, name=f"vps{s}", tag="sm", bufs=4)
                for c in range(4):
                    nc.tensor.matmul(
                        out=vps,
                        lhsT=xn[c][:, s * 128 : (s + 1) * 128],
                        rhs=wqkvT[c][:, 2 * C : 3 * C],
                        start=(c == 0),
                        stop=(c == 3),
                    )
                nc.vector.tensor_tensor(
                    out=vt_s[:, :, 0:HD],
                    in0=vps.rearrange("p (h d) -> p h d", d=HD),
                    in1=bvb.rearrange("p (h d) -> p h d", d=HD),
                    op=ALU.add,
                )
                vT.append(vt_s)

            # ---- Q / K o-tile pairs + head pairs ----
            q_sb = [None] * 4
            k_sb = [None] * 4

            def emit_qk_pair(jt: int):
                for which, (store, col0, btiles) in enumerate(
                    ((q_sb, 0, bqk[0:4]), (k_sb, C, bqk[4:8]))
                ):
                    dst = pp.tile(
                        [128, T], BF16, name=f"{'qk'[which]}{jt}", tag=f"{'qk'[which]}{jt}"
                    )
                    for tb in range(2):
                        ps = pool_ps.tile([128, 512], F32, name="qkps", tag="sm", bufs=4)
                        for c in range(4):
                            nc.tensor.matmul(
                                out=ps,
                                lhsT=wqkvT[c][
                                    :, col0 + jt * 128 : col0 + (jt + 1) * 128
                                ],
                                rhs=xn[c][:, tb * 512 : (tb + 1) * 512],
                                start=(c == 0),
                                stop=(c == 3),
                            )
                        nc.vector.tensor_scalar_add(
                            dst[:, tb * 512 : (tb + 1) * 512], ps, btiles[jt]
                        )
                    store[jt] = dst

            def emit_head_pair(p: int):
                jt = p
                # S^T + exp for both heads, interleaved (row-group packing on PE)
                expS = {0: [], 1: []}
                for s in range(8):
                    sps = {}
                    for hh in range(2):
                        off = 64 * hh
                        sp = pool_ps.tile([128, T], F32, name="sps", tag="st", bufs=2)
                        for tb in range(2):
                            nc.tensor.matmul(
                                out=sp[:, tb * 512 : (tb + 1) * 512],
                                lhsT=k_sb[jt][
                                    off : off + 64, s * 128 : (s + 1) * 128
                                ],
                                rhs=q_sb[jt][
                                    off : off + 64, tb * 512 : (tb + 1) * 512
                                ],
                                start=True,
                                stop=True,
                            )
                        sps[hh] = sp
                    for hh in range(2):
                        es = wp.tile([128, T], BF16, name="expS", tag="expS", bufs=32)
                        nc.scalar.activation(out=es, in_=sps[hh], func=ACTF.Exp)
                        expS[hh].append(es)
                return expS

            def emit_av_pair(p: int, expS) -> None:
                jt = p
                for hh in range(2):
                    h = 2 * p + hh
                    off = 64 * hh
                    for tb in range(2):
                        hps = pool_ps.tile([HD + 1, 512], F32, name="hps", tag="sm", bufs=4)
                        for s in range(8):
                            nc.tensor.matmul(
                                out=hps,
                                lhsT=vT[s][:, h, :],
                                rhs=expS[hh][s][:, tb * 512 : (tb + 1) * 512],
                                start=(s == 0),
                                stop=(s == 7),
                            )
                        rrow = wp.tile([1, 512], F32, name="rrow", tag="rrow", bufs=2)
                        nc.vector.reciprocal(out=rrow, in_=hps[HD : HD + 1, :])
                        rb = wp.tile([64, 512], F32, name="rb", tag="rb", bufs=2)
                        nc.gpsimd.partition_broadcast(out_ap=rb, in_ap=rrow, channels=64)
                        if off == 0:
                            nc.vector.tensor_tensor(
                                out=hn_sb[jt][0:64, tb * 512 : (tb + 1) * 512],
                                in0=hps[0:HD, :],
                                in1=rb,
                                op=ALU.mult,
                            )
                        else:
                            hstg = wp.tile([64, 512], BF16, name="hstg", tag="hstg", bufs=2)
                            nc.vector.tensor_tensor(
                                out=hstg, in0=hps[0:HD, :], in1=rb, op=ALU.mult
                            )
                            nc.sync.dma_start(
                                out=hn_sb[jt][64:128, tb * 512 : (tb + 1) * 512],
                                in_=hstg,
                            )

            hn_sb = []
            for j in range(4):
                hn_j = pp.tile([128, T], BF16, name=f"hn{j}", tag=f"hn{j}")
                hn_sb.append(hn_j)

            # software pipeline over head pairs, QK matmuls interleaved to keep
            # the PE busy while ACT runs the exps
            emit_qk_pair(0)
            prev = None  # (pair_idx, expS)
            for p in range(4):
                expS = emit_head_pair(p)
                if p < 3:
                    emit_qk_pair(p + 1)
                if prev is not None:
                    emit_av_pair(*prev)
                prev = (p, expS)
            emit_av_pair(*prev)

            # ---- projection + bias + residual ----
            for o in range(4):
                for tb in range(2):
                    pps = pool_ps.tile([128, 512], F32, name="pps", tag="sm", bufs=4)
                    for c in range(4):
                        nc.tensor.matmul(
                            out=pps,
                            lhsT=wprojT[c][:, o * 128 : (o + 1) * 128],
                            rhs=hn_sb[c][:, tb * 512 : (tb + 1) * 512],
                            start=(c == 0),
                            stop=(c == 3),
                        )
                    ot = wp.tile([128, 512], F32, name="ot", tag="ot", bufs=2)
                    nc.vector.scalar_tensor_tensor(
                        out=ot,
                        in0=pps,
                        scalar=bproj[o],
                        in1=xt[o][:, tb * 512 : (tb + 1) * 512],
                        op0=ALU.add,
                        op1=ALU.add,
                    )
                    nc.sync.dma_start(
                        out=out_r[o][:, tb * 512 : (tb + 1) * 512], in_=ot
                    )


_NC_CACHE = None


def build_nc() -> bacc.Bacc:
    global _NC_CACHE
    if _NC_CACHE is None:
        nc = bacc.Bacc("TRN2", target_bir_lowering=False, debug=False, num_devices=B)
        _emit(nc)
        nc.compile()
        _NC_CACHE = nc
    return _NC_CACHE


def prep_inputs(x, gn_w, gn_b, qkv_w, qkv_b, proj_w, proj_b):
    """Host-side reformat: returns the per-core in_map dicts (core i = batch i)."""
    x = np.ascontiguousarray(np.asarray(x, dtype=np.float32))
    gn_w = np.asarray(gn_w, dtype=np.float32)
    gn_b = np.asarray(gn_b, dtype=np.float32)
    qkv_w = np.asarray(qkv_w, dtype=np.float32)
    qkv_b = np.asarray(qkv_b, dtype=np.float32)
    proj_w = np.asarray(proj_w, dtype=np.float32)
    proj_b = np.asarray(proj_b, dtype=np.float32)

    scale = float(HD) ** -0.25
    idx_q = np.concatenate([np.arange(3 * HD * h, 3 * HD * h + HD) for h in range(HEADS)])
    idx_k = idx_q + HD
    idx_v = idx_q + 2 * HD
    wq = qkv_w[idx_q] * scale
    wk = qkv_w[idx_k] * scale
    wv = qkv_w[idx_v]
    import ml_dtypes

    wqkvT = np.ascontiguousarray(
        np.concatenate([wq, wk, wv], axis=0).T.astype(ml_dtypes.bfloat16)
    )  # (512, 1536) bf16
    bqk = np.concatenate([qkv_b[idx_q] * scale, qkv_b[idx_k] * scale]).reshape(2 * C, 1)
    bv_row = np.ascontiguousarray(qkv_b[idx_v].reshape(1, C))
    wprojT = np.ascontiguousarray(proj_w.T.astype(ml_dtypes.bfloat16))
    bproj = proj_b.reshape(C, 1)

    gmat = np.zeros((4, 128, G), dtype=np.float32)
    gmatT = np.zeros((4, G, 128), dtype=np.float32)
    for j in range(4):
        for cl in range(128):
            g = 8 * j + cl // GSIZE
            gmat[j, cl, g] = 1.0
            gmatT[j, g, cl] = 1.0

    shared = {
        "wqkvT": wqkvT,
        "bqk": np.ascontiguousarray(bqk),
        "bv_row": bv_row,
        "gnw": np.ascontiguousarray(gn_w.reshape(C, 1)),
        "gnb": np.ascontiguousarray(gn_b.reshape(C, 1)),
        "wprojT": wprojT,
        "bproj": np.ascontiguousarray(bproj),
        "gmat": gmat,
        "gmatT": gmatT,
    }
    in_maps = []
    for b in range(B):
        m = {"x": np.ascontiguousarray(x[b].reshape(C, T))}
        m.update(shared)
        in_maps.append(m)
    return in_maps


def kernel(x, gn_w, gn_b, qkv_w, qkv_b, proj_w, proj_b):
    in_maps = prep_inputs(x, gn_w, gn_b, qkv_w, qkv_b, proj_w, proj_b)
    nc = build_nc()
    res = run_bass_kernel_spmd(nc, in_maps, core_ids=list(range(B)))
    out = np.stack([res.results[i]["out"] for i in range(B)], axis=0)
    return out.reshape(B, C, 32, 32).astype(np.float32)


# revision 7
# speedup vs baseline: 13.4969x; 13.4969x over previous
"""Trainium2 Bass kernel for nn_AttentionBlock (GroupNorm + 8-head self-attention
+ projection + residual) on input x:(8,512,32,32) f32.

Strategy: pure data-parallel over batch — each of the 8 NeuronCores processes
one batch element end-to-end (no collectives). Per core:

  x (512,1024) --GroupNorm--> xn --qkv matmul--> Q,K (o-part,t-free), V^T (s-part,c-free)
  per head h: S^T = K_h^T Q_h (s-part, t-free); expS = exp(S^T) [no max-sub; |S|<7]
              H_ext = [V_h^T | 1]^T expS  (rows 0..63 = unnormalized AV, row 64 = softmax denom)
              H = H_ext[0:64] * recip(H_ext[64]) broadcast (gpsimd partition_broadcast)
  out = proj @ H + bproj + x

All big matmuls run in bf16 (full PE rate; measured ~3e-3 absmax error vs the
fp32 reference, reference absmax ~5.4). Softmax scale and the per-head q/k/v
row split are folded into the qkv weights on the host, which also
pre-transposes and pre-casts the weights.

The head loop is emitted as a software pipeline: per head-pair, the S^T
matmuls + exps stream while the PE's program order is filled with the
previous pair's AV matmuls and the next pair's QK matmuls, so the PE never
waits on the ScalarEngine's exp throughput (the ~73us/core floor).
"""

import numpy as np

import concourse.bacc as bacc
import concourse.bass2jax as bass2jax
import concourse.mybir as mybir
import concourse.tile as tile
from concourse.bass_utils import run_bass_kernel_spmd


def _install_neff_disk_cache():
    """Wrap bass2jax.neuronx_cc_hook with a content-addressed on-disk cache so
    repeated processes skip the multi-minute walrus compile of an identical
    kernel. Keyed on the HLO bytes (which embed the serialized BIR)."""
    if getattr(bass2jax, "_ant_neff_disk_cache", False):
        return
    import hashlib
    import os

    cache_dir = os.environ.get("BASS_NEFF_CACHE", "/tmp/bass_neff_cache")
    try:
        os.makedirs(cache_dir, exist_ok=True)
    except OSError:
        return
    orig = bass2jax.neuronx_cc_hook

    def cached_hook(code, code_format, platform_version, file_prefix):
        key = None
        if isinstance(code, (bytes, bytearray)) and b"bass_exec" in code:
            key = hashlib.sha256(bytes(code)).hexdigest()
            path = os.path.join(cache_dir, key + ".neffcc")
            if os.path.exists(path):
                with open(path, "rb") as f:
                    return 0, f.read()
        r = orig(code, code_format, platform_version, file_prefix)
        if key is not None:
            try:
                status, data = r
                if status == 0 and isinstance(data, (bytes, bytearray)):
                    tmp = path + f".tmp{os.getpid()}"
                    with open(tmp, "wb") as f:
                        f.write(bytes(data))
                    os.replace(tmp, path)
            except Exception:
                pass
        return r

    bass2jax.neuronx_cc_hook = cached_hook
    bass2jax._ant_neff_disk_cache = True


_install_neff_disk_cache()

B = 8
C = 512
T = 1024
HEADS = 8
HD = 64  # head dim
G = 32  # groupnorm groups
GSIZE = C // G  # 16 channels per group
EPS = 1e-5

F32 = mybir.dt.float32
BF16 = mybir.dt.bfloat16
AX = mybir.AxisListType
ALU = mybir.AluOpType
ACTF = mybir.ActivationFunctionType


def _emit_iter(nc, pp, wp, pool_ps, dram):
    """One full attention-block iteration (per core = one batch element)."""
    x_r = dram["x"].rearrange("(j p) t -> j p t", p=128)
    wqkvT_r = dram["wqkvT"].rearrange("(j p) o -> j p o", p=128)
    bqk_r = dram["bqk"].rearrange("(j p) o -> j p o", p=128)
    gnw_r = dram["gnw"].rearrange("(j p) o -> j p o", p=128)
    gnb_r = dram["gnb"].rearrange("(j p) o -> j p o", p=128)
    wprojT_r = dram["wprojT"].rearrange("(j p) o -> j p o", p=128)
    bproj_r = dram["bproj"].rearrange("(j p) o -> j p o", p=128)
    out_r = dram["out"].rearrange("(j p) t -> j p t", p=128)

    # ---- constant / weight loads ----
    xt = []
    for j in range(4):
        x_sb = pp.tile([128, T], F32, name=f"x{j}", tag=f"x{j}")
        nc.sync.dma_start(out=x_sb, in_=x_r[j])
        xt.append(x_sb)
    wqkvT = []
    for j in range(4):
        wq_sb = pp.tile([128, 3 * C], BF16, name=f"wqkvT{j}", tag=f"wqkvT{j}")
        nc.sync.dma_start(out=wq_sb, in_=wqkvT_r[j])
        wqkvT.append(wq_sb)
    wprojT = []
    for j in range(4):
        wp_sb = pp.tile([128, C], BF16, name=f"wprojT{j}", tag=f"wprojT{j}")
        nc.sync.dma_start(out=wp_sb, in_=wprojT_r[j])
        wprojT.append(wp_sb)
    bqk = []
    for j in range(8):
        b_sb = pp.tile([128, 1], F32, name=f"bqk{j}", tag=f"bqk{j}")
        nc.sync.dma_start(out=b_sb, in_=bqk_r[j])
        bqk.append(b_sb)
    bproj = []
    for j in range(4):
        bp_sb = pp.tile([128, 1], F32, name=f"bproj{j}", tag=f"bproj{j}")
        nc.sync.dma_start(out=bp_sb, in_=bproj_r[j])
        bproj.append(bp_sb)
    gnw, gnb = [], []
    for j in range(4):
        gw_sb = pp.tile([128, 1], F32, name=f"gnw{j}", tag=f"gnw{j}")
        nc.sync.dma_start(out=gw_sb, in_=gnw_r[j])
        gnw.append(gw_sb)
        gb_sb = pp.tile([128, 1], F32, name=f"gnb{j}", tag=f"gnb{j}")
        nc.sync.dma_start(out=gb_sb, in_=gnb_r[j])
        gnb.append(gb_sb)
    gmat, gmatT = [], []
    for j in range(4):
        gm_sb = pp.tile([128, G], F32, name=f"gmat{j}", tag=f"gmat{j}")
        nc.sync.dma_start(out=gm_sb, in_=dram["gmat"][j])
        gmat.append(gm_sb)
        gmt_sb = pp.tile([G, 128], F32, name=f"gmatT{j}", tag=f"gmatT{j}")
        nc.sync.dma_start(out=gmt_sb, in_=dram["gmatT"][j])
        gmatT.append(gmt_sb)
    bvrow = pp.tile([1, C], F32, name="bvrow", tag="bvrow")
    nc.sync.dma_start(out=bvrow, in_=dram["bv_row"])
    bvb = pp.tile([128, C], F32, name="bvb", tag="bvb")
    nc.gpsimd.partition_broadcast(out_ap=bvb, in_ap=bvrow, channels=128)

    # ---- GroupNorm ----
    stats = []
    for j in range(4):
        stat = pp.tile([128, 2], F32, name=f"stat{j}", tag=f"stat{j}")
        nc.vector.reduce_sum(stat[:, 0:1], xt[j], axis=AX.X)
        scr = wp.tile([128, T], F32, name="sqscr", tag="sqscr", bufs=1)
        nc.vector.scalar_tensor_tensor(
            out=scr,
            in0=xt[j],
            scalar=1.0,
            in1=xt[j],
            op0=ALU.mult,
            op1=ALU.mult,
            accum_out=stat[:, 1:2],
        )
        stats.append(stat)
    gsum = pool_ps.tile([G, 2], F32, name="gsum", tag="sm", bufs=4)
    for j in range(4):
        nc.tensor.matmul(
            out=gsum, lhsT=gmat[j], rhs=stats[j], start=(j == 0), stop=(j == 3)
        )
    gstat = pp.tile([G, 2], F32, name="gstat", tag="gstat")
    nc.vector.tensor_scalar_mul(gstat, gsum, 1.0 / float(GSIZE * T))
    m2 = pp.tile([G, 1], F32, name="m2", tag="m2")
    nc.vector.tensor_tensor(out=m2, in0=gstat[:, 0:1], in1=gstat[:, 0:1], op=ALU.mult)
    var = pp.tile([G, 1], F32, name="var", tag="var")
    nc.vector.tensor_tensor(out=var, in0=gstat[:, 1:2], in1=m2, op=ALU.subtract)
    nc.vector.tensor_scalar_add(var, var, EPS)
    std = pp.tile([G, 1], F32, name="std", tag="std")
    nc.scalar.activation(out=std, in_=var, func=ACTF.Sqrt, bias=0.0)
    grs = pp.tile([G, 2], F32, name="grs", tag="grs")
    nc.vector.tensor_copy(out=grs[:, 0:1], in_=gstat[:, 0:1])
    nc.vector.reciprocal(out=grs[:, 1:2], in_=std)

    xn = []
    for j in range(4):
        chs = pool_ps.tile([128, 2], F32, name=f"chs{j}", tag="sm", bufs=4)
        nc.tensor.matmul(out=chs, lhsT=gmatT[j], rhs=grs, start=True, stop=True)
        a_j = pp.tile([128, 1], F32, name=f"a{j}", tag=f"a{j}")
        nc.vector.tensor_tensor(out=a_j, in0=gnw[j], in1=chs[:, 1:2], op=ALU.mult)
        nb = wp.tile([128, 1], F32, name="nb", tag="nb")
        nc.vector.tensor_tensor(out=nb, in0=chs[:, 0:1], in1=a_j, op=ALU.mult)
        b_j = pp.tile([128, 1], F32, name=f"b{j}", tag=f"b{j}")
        nc.vector.tensor_tensor(out=b_j, in0=gnb[j], in1=nb, op=ALU.subtract)
        xn_j = pp.tile([128, T], BF16, name=f"xn{j}", tag=f"xn{j}")
        nc.vector.tensor_scalar(
            out=xn_j, in0=xt[j], scalar1=a_j, scalar2=b_j, op0=ALU.mult, op1=ALU.add
        )
        xn.append(xn_j)

    # ---- pipelined main loop ----
    q_sb = [None] * 4
    k_sb = [None] * 4
    hn_sb = []
    for j in range(4):
        hn_j = pp.tile([128, T], BF16, name=f"hn{j}", tag=f"hn{j}")
        hn_sb.append(hn_j)
    vT = [None] * 8

    def emit_vt_chunk(s: int) -> None:
        """V^T s-tile: V^T[s,:] for all heads (+ ones col), 4 matmuls."""
        vt_s = pp.tile([128, HEADS, HD + 1], BF16, name=f"vT{s}", tag=f"vT{s}")
        nc.vector.memset(vt_s[:, :, HD : HD + 1], 1.0)
        vps = pool_ps.tile([128, C], F32, name=f"vps{s}", tag="sm", bufs=4)
        for c in range(4):
            nc.tensor.matmul(
                out=vps,
                lhsT=xn[c][:, s * 128 : (s + 1) * 128],
                rhs=wqkvT[c][:, 2 * C : 3 * C],
                start=(c == 0),
                stop=(c == 3),
            )
        nc.vector.tensor_tensor(
            out=vt_s[:, :, 0:HD],
            in0=vps.rearrange("p (h d) -> p h d", d=HD),
            in1=bvb.rearrange("p (h d) -> p h d", d=HD),
            op=ALU.add,
        )
        vT[s] = vt_s

    def make_qk_chunks(jt: int):
        """QK o-tile pair jt as 8 chunks of 2 matmuls (4 groups x 4 c-mms)."""
        dsts = {}
        for which in range(2):
            dsts[which] = pp.tile(
                [128, T], BF16, name=f"{'qk'[which]}{jt}", tag=f"{'qk'[which]}{jt}"
            )
        state = {}

        def chunk(s: int) -> None:
            grp = s // 2  # 0..3: (which, tb)
            which, tb = grp // 2, grp % 2
            col0 = which * C
            if s % 2 == 0:
                state["ps"] = pool_ps.tile([128, 512], F32, name="qkps", tag="sm", bufs=4)
            ps = state["ps"]
            for c in (2 * (s % 2), 2 * (s % 2) + 1):
                nc.tensor.matmul(
                    out=ps,
                    lhsT=wqkvT[c][:, col0 + jt * 128 : col0 + (jt + 1) * 128],
                    rhs=xn[c][:, tb * 512 : (tb + 1) * 512],
                    start=(c == 0),
                    stop=(c == 3),
                )
            if s % 2 == 1:
                bias = bqk[which * 4 + jt]
                nc.vector.tensor_scalar_add(
                    dsts[which][:, tb * 512 : (tb + 1) * 512], ps, bias
                )

        def finish():
            q_sb[jt] = dsts[0]
            k_sb[jt] = dsts[1]

        return chunk, finish

    def emit_s_exp(p: int, s: int, expS) -> None:
        """S^T matmuls + exp for head pair p, s-block s (both heads)."""
        jt = p
        sps = {}
        for hh in range(2):
            off = 64 * hh
            sp = pool_ps.tile([128, T], F32, name="sps", tag="st", bufs=2)
            for tb in range(2):
                nc.tensor.matmul(
                    out=sp[:, tb * 512 : (tb + 1) * 512],
                    lhsT=k_sb[jt][off : off + 64, s * 128 : (s + 1) * 128],
                    rhs=q_sb[jt][off : off + 64, tb * 512 : (tb + 1) * 512],
                    start=True,
                    stop=True,
                )
            sps[hh] = sp
        for hh in range(2):
            es = wp.tile([128, T], BF16, name="expS", tag="expS", bufs=32)
            nc.scalar.activation(out=es, in_=sps[hh], func=ACTF.Exp)
            expS[hh].append(es)

    def make_av_chunks(p: int, expS):
        """AV + normalize for head pair p as 8 chunks of 4 matmuls."""
        jt = p
        state = {}

        def chunk(s: int) -> None:
            grp = s // 2  # (hh, tb)
            hh, tb = grp // 2, grp % 2
            h = 2 * p + hh
            if s % 2 == 0:
                state["ps"] = pool_ps.tile(
                    [HD + 1, 512], F32, name="hps", tag="sm", bufs=4
                )
            hps = state["ps"]
            s0 = 4 * (s % 2)
            for si in range(s0, s0 + 4):
                nc.tensor.matmul(
                    out=hps,
                    lhsT=vT[si][:, h, :],
                    rhs=expS[hh][si][:, tb * 512 : (tb + 1) * 512],
                    start=(si == 0),
                    stop=(si == 7),
                )
            if s % 2 == 1:
                off = 64 * hh
                rrow = wp.tile([1, 512], F32, name="rrow", tag="rrow", bufs=2)
                nc.vector.reciprocal(out=rrow, in_=hps[HD : HD + 1, :])
                rb = wp.tile([64, 512], F32, name="rb", tag="rb", bufs=2)
                nc.gpsimd.partition_broadcast(out_ap=rb, in_ap=rrow, channels=64)
                if off == 0:
                    nc.vector.tensor_tensor(
                        out=hn_sb[jt][0:64, tb * 512 : (tb + 1) * 512],
                        in0=hps[0:HD, :],
                        in1=rb,
                        op=ALU.mult,
                    )
                else:
                    hstg = wp.tile([64, 512], BF16, name="hstg", tag="hstg", bufs=2)
                    nc.vector.tensor_tensor(
                        out=hstg, in0=hps[0:HD, :], in1=rb, op=ALU.mult
                    )
                    nc.sync.dma_start(
                        out=hn_sb[jt][64:128, tb * 512 : (tb + 1) * 512], in_=hstg
                    )

        return chunk

    # prologue: QK pair 0 in full
    qk_chunk, qk_finish = make_qk_chunks(0)
    for s in range(8):
        qk_chunk(s)
    qk_finish()

    av_chunk = None
    for p in range(4):
        expS = {0: [], 1: []}
        if p < 3:
            qk_chunk, qk_finish = make_qk_chunks(p + 1)
        else:
            qk_chunk, qk_finish = None, None
        for s in range(8):
            emit_s_exp(p, s, expS)
            if p == 0:
                emit_vt_chunk(s)
            if av_chunk is not None:
                av_chunk(s)
            if qk_chunk is not None:
                qk_chunk(s)
        if qk_finish is not None:
            qk_finish()
        av_chunk = make_av_chunks(p, expS)
    for s in range(8):
        av_chunk(s)

    # ---- projection + bias + residual ----
    for o in range(4):
        for tb in range(2):
            pps = pool_ps.tile([128, 512], F32, name="pps", tag="sm", bufs=4)
            for c in range(4):
                nc.tensor.matmul(
                    out=pps,
                    lhsT=wprojT[c][:, o * 128 : (o + 1) * 128],
                    rhs=hn_sb[c][:, tb * 512 : (tb + 1) * 512],
                    start=(c == 0),
                    stop=(c == 3),
                )
            ot = wp.tile([128, 512], F32, name="ot", tag="ot", bufs=4)
            nc.vector.scalar_tensor_tensor(
                out=ot,
                in0=pps,
                scalar=bproj[o],
                in1=xt[o][:, tb * 512 : (tb + 1) * 512],
                op0=ALU.add,
                op1=ALU.add,
            )
            nc.sync.dma_start(out=out_r[o][:, tb * 512 : (tb + 1) * 512], in_=ot)


def _emit(nc, repeats: int = 1) -> None:
    dram = {
        "x": nc.dram_tensor("x", [C, T], F32, kind="ExternalInput").ap(),
        "wqkvT": nc.dram_tensor("wqkvT", [C, 3 * C], BF16, kind="ExternalInput").ap(),
        "bqk": nc.dram_tensor("bqk", [2 * C, 1], F32, kind="ExternalInput").ap(),
        "bv_row": nc.dram_tensor("bv_row", [1, C], F32, kind="ExternalInput").ap(),
        "gnw": nc.dram_tensor("gnw", [C, 1], F32, kind="ExternalInput").ap(),
        "gnb": nc.dram_tensor("gnb", [C, 1], F32, kind="ExternalInput").ap(),
        "wprojT": nc.dram_tensor("wprojT", [C, C], BF16, kind="ExternalInput").ap(),
        "bproj": nc.dram_tensor("bproj", [C, 1], F32, kind="ExternalInput").ap(),
        "gmat": nc.dram_tensor("gmat", [4, 128, G], F32, kind="ExternalInput").ap(),
        "gmatT": nc.dram_tensor("gmatT", [4, G, 128], F32, kind="ExternalInput").ap(),
        "out": nc.dram_tensor("out", [C, T], F32, kind="ExternalOutput").ap(),
    }
    with tile.TileContext(nc) as tc:
        with (
            tc.tile_pool(name="persist", bufs=1) as pp,
            tc.tile_pool(name="work", bufs=2) as wp,
            tc.tile_pool(name="psum", bufs=1, space="PSUM") as pool_ps,
        ):
            for _ in range(repeats):
                _emit_iter(nc, pp, wp, pool_ps, dram)


_NC_CACHE = {}


def build_nc(repeats: int = 1):
    if repeats not in _NC_CACHE:
        nc = bacc.Bacc("TRN2", target_bir_lowering=False, debug=False, num_devices=B)
        _emit(nc, repeats=repeats)
        nc.compile()
        _NC_CACHE[repeats] = nc
    return _NC_CACHE[repeats]


def prep_inputs(x, gn_w, gn_b, qkv_w, qkv_b, proj_w, proj_b):
    """Host-side reformat: returns the per-core in_map dicts (core i = batch i)."""
    import ml_dtypes

    x = np.ascontiguousarray(np.asarray(x, dtype=np.float32))
    gn_w = np.asarray(gn_w, dtype=np.float32)
    gn_b = np.asarray(gn_b, dtype=np.float32)
    qkv_w = np.asarray(qkv_w, dtype=np.float32)
    qkv_b = np.asarray(qkv_b, dtype=np.float32)
    proj_w = np.asarray(proj_w, dtype=np.float32)
    proj_b = np.asarray(proj_b, dtype=np.float32)

    scale = float(HD) ** -0.25
    idx_q = np.concatenate([np.arange(3 * HD * h, 3 * HD * h + HD) for h in range(HEADS)])
    idx_k = idx_q + HD
    idx_v = idx_q + 2 * HD
    wq = qkv_w[idx_q] * scale
    wk = qkv_w[idx_k] * scale
    wv = qkv_w[idx_v]
    wqkvT = np.ascontiguousarray(
        np.concatenate([wq, wk, wv], axis=0).T.astype(ml_dtypes.bfloat16)
    )  # (512, 1536) bf16
    bqk = np.concatenate([qkv_b[idx_q] * scale, qkv_b[idx_k] * scale]).reshape(2 * C, 1)
    bv_row = np.ascontiguousarray(qkv_b[idx_v].reshape(1, C))
    wprojT = np.ascontiguousarray(proj_w.T.astype(ml_dtypes.bfloat16))
    bproj = proj_b.reshape(C, 1)

    gmat = np.zeros((4, 128, G), dtype=np.float32)
    gmatT = np.zeros((4, G, 128), dtype=np.float32)
    for j in range(4):
        for cl in range(128):
            g = 8 * j + cl // GSIZE
            gmat[j, cl, g] = 1.0
            gmatT[j, g, cl] = 1.0

    shared = {
        "wqkvT": wqkvT,
        "bqk": np.ascontiguousarray(bqk),
        "bv_row": bv_row,
        "gnw": np.ascontiguousarray(gn_w.reshape(C, 1)),
        "gnb": np.ascontiguousarray(gn_b.reshape(C, 1)),
        "wprojT": wprojT,
        "bproj": np.ascontiguousarray(bproj),
        "gmat": gmat,
        "gmatT": gmatT,
    }
    in_maps = []
    for b in range(B):
        m = {"x": np.ascontiguousarray(x[b].reshape(C, T))}
        m.update(shared)
        in_maps.append(m)
    return in_maps


def kernel(x, gn_w, gn_b, qkv_w, qkv_b, proj_w, proj_b):
    in_maps = prep_inputs(x, gn_w, gn_b, qkv_w, qkv_b, proj_w, proj_b)
    nc = build_nc()
    res = run_bass_kernel_spmd(nc, in_maps, core_ids=list(range(B)))
    out = np.stack([res.results[i]["out"] for i in range(B)], axis=0)
    return out.reshape(B, C, 32, 32).astype(np.float32)
